# revision 2
# baseline (speedup 1.0000x reference)
"""3-layer GAT (PyG GATConv semantics) on 8 TRN2 NeuronCores — v2.

Versus the baseline: all tables/gathers/matmuls in bf16 (halves PJRT upload
and HBM traffic), and the per-chunk DVE work is batched per 128-dst window
with broadcast access patterns:
  - one is_equal op builds all CPW one-hot S blocks of a window at once
  - per-edge a_dst comes from a replicate-matmul + fused mult-reduce
    (no per-chunk transposes / selection matmuls)
  - alpha is applied to the gathered rows with one broadcast multiply; the
    softmax denominator rides along as a ones-column through the same
    aggregation matmuls.
Softmax uses shift-invariance (logits clamped at 60; no segment max).
"""
import numpy as np
import sys

sys.path.insert(0, "/opt/trn_rl_repo")
from concourse import bass, mybir, bacc  # noqa: E402
import concourse.tile as tile  # noqa: E402
from concourse import bass_utils  # noqa: E402
from concourse.masks import make_identity  # noqa: E402

try:
    import ml_dtypes
    BF16NP = np.dtype(ml_dtypes.bfloat16)
except Exception:  # pragma: no cover
    BF16NP = None

F32 = mybir.dt.float32
BF16 = mybir.dt.bfloat16
I32 = mybir.dt.int32
AF = mybir.ActivationFunctionType
ALU = mybir.AluOpType

N, E_EDGES = 100_000, 1_600_000
IN, HID, H, OUT = 256, 32, 4, 40
NC = 8

_CACHE = {}


def _host_prep(edge, n, ncores):
    nd = n // ncores
    ndp = ((nd + 127) // 128) * 128
    nw = ndp // 128
    np_tot = ncores * ndp

    src = np.concatenate([edge[0], np.arange(n, dtype=np.int64)])
    dst = np.concatenate([edge[1], np.arange(n, dtype=np.int64)])
    core = dst // nd
    gsrc = (src // nd) * ndp + (src % nd)

    per_core = []
    cpw = 1
    for k in range(ncores):
        m = core == k
        s = gsrc[m]
        d = dst[m] - k * nd
        w = d // 128
        order = np.lexsort((d, w))
        s, d, w = s[order], d[order], w[order]
        cnt = np.bincount(w.astype(np.int64), minlength=nw)
        cpw = max(cpw, int(np.max((cnt + 127) // 128)))
        per_core.append((s, d, cnt))

    cores = []
    for k in range(ncores):
        s, d, cnt = per_core[k]
        idx32 = np.zeros((nw * cpw, 128), np.int64)
        drel = np.full((nw * cpw, 128), -1.0, np.float32)
        off = 0
        for wi in range(nw):
            cn = int(cnt[wi])
            bs = s[off:off + cn]
            bd = d[off:off + cn] - wi * 128
            off += cn
            c0 = wi * cpw
            idx32[c0:c0 + cpw].reshape(-1)[:cn] = bs
            drel[c0:c0 + cpw].reshape(-1)[:cn] = bd
        cores.append(dict(
            idx32=idx32.T.astype(np.int32).copy(),
            drel_col=drel.T.astype(np.float32).copy()))
    shapes = dict(ND=nd, NDP=ndp, NW=nw, NP_TOT=np_tot, CPW=cpw,
                  NCH=nw * cpw)
    return cores, shapes


def _pack_weights(W1, a_src1, a_dst1, W2, a_src2, a_dst2, W3, a_src3,
                  a_dst3):
    """Augment each W with per-head a_dst columns then a_src columns:
    table row = [xh (HD) | ad (H) | as (H) | one]  (ones col written on
    device).  Row widths: L1 = 128+4+4+1 -> pad 144; L2 = 128+1+1+1 -> 132;
    L3 = 40+1+1+1 -> pad 48."""
    HD = HID * H

    def aug(W, a_dst, a_src, heads, hid, pad):
        cd = [W[:, h * hid:(h + 1) * hid] @ a_dst[h] for h in range(heads)]
        cs = [W[:, h * hid:(h + 1) * hid] @ a_src[h] for h in range(heads)]
        out = np.concatenate(
            [W] + [c[:, None] for c in cd] + [c[:, None] for c in cs], 1)
        full = np.zeros((W.shape[0], pad), np.float32)
        full[:, :out.shape[1]] = out
        return full

    W1a = aug(W1, a_dst1, a_src1, H, HID, 144)
    W2a = aug(W2, a_dst2, a_src2, 1, HD, 132)
    W3a = aug(W3, a_dst3, a_src3, 1, OUT, 48)
    return W1a, W2a, W3a


def _build_kernel(shapes):
    NDP, NW, NP, CPW, NCH = (shapes[x] for x in
                             ("NDP", "NW", "NP_TOT", "CPW", "NCH"))
    HD = HID * H
    KT = IN // 128

    # per-layer table geometry: row = [xh | ad(H) | as(H) | one | pad]
    TC1, TC2, TC3 = 144, 132, 48
    XC1, XC2, XC3 = HD, HD, OUT

    nc = bacc.Bacc("TRN2", target_bir_lowering=False, debug=False,
                   enable_asserts=False, num_devices=NC)
    dt = nc.dram_tensor
    # single packed input blob (uint16 units viewed as bf16); per-tensor
    # PJRT transfers cost ~0.15s each, so ship ONE tensor.
    L_XT = IN * NDP
    L_W1 = IN * TC1
    L_W2 = HD * TC2
    L_W3 = HD * TC3
    L_DR = 128 * NCH
    L_IX = 128 * NCH * 2
    TOT = L_XT + L_W1 + L_W2 + L_W3 + L_DR + L_IX
    assert TOT % 128 == 0
    blob2 = dt("blob", [128, TOT // 128], BF16, kind="ExternalInput").ap()
    blob = blob2.rearrange("(o p) q -> o (p q)", o=1)

    def view(off, ln, r, c):
        return blob[0:1, off:off + ln].rearrange(
            "o (r c) -> (o r) c", r=r, c=c)

    o = 0
    xT = view(o, L_XT, IN, NDP); o += L_XT
    w1 = view(o, L_W1, IN, TC1); o += L_W1
    w2 = view(o, L_W2, HD, TC2); o += L_W2
    w3 = view(o, L_W3, HD, TC3); o += L_W3
    drel_c = view(o, L_DR, 128, NCH); o += L_DR
    idx32 = blob[0:1, o:o + L_IX].bitcast(I32).rearrange(
        "o (r c) -> (o r) c", r=128, c=NCH)
    out = dt("out", [NDP, OUT], mybir.dt.float16,
             kind="ExternalOutput").ap()

    with tile.TileContext(nc) as tc:
        with tc.tile_pool(name="const", bufs=1) as cpool, \
             tc.tile_pool(name="dense", bufs=3) as dpool, \
             tc.tile_pool(name="win", bufs=3) as wpool, \
             tc.tile_pool(name="small", bufs=3) as spool, \
             tc.tile_pool(name="psum", bufs=2, space="PSUM") as pp, \
             tc.tile_pool(name="psum_ad", bufs=2, space="PSUM") as pad_p, \
             tc.tile_pool(name="psum_tp", bufs=2, space="PSUM") as ptp, \
             tc.tile_pool(name="dram", bufs=1, space="DRAM") as dram:

            ident = cpool.tile([128, 128], F32)
            make_identity(nc, ident[:])
            ident_b = cpool.tile([128, 128], BF16)
            nc.vector.tensor_copy(ident_b[:], ident[:])
            iota_i = cpool.tile([128, 128], I32)
            nc.gpsimd.iota(iota_i[:], pattern=[[1, 128]], base=0,
                           channel_multiplier=0)
            iota_row = cpool.tile([128, 128], F32)
            nc.vector.tensor_copy(iota_row[:], iota_i[:])
            # one-hot head selectors: OH[p, h*128+e] = (p == h)
            oh_i = cpool.tile([128, H * 128], I32)
            nc.gpsimd.iota(oh_i[:], pattern=[[-1, H], [0, 128]], base=0,
                           channel_multiplier=1)
            oh_f = cpool.tile([128, H * 128], F32)
            nc.vector.tensor_copy(oh_f[:], oh_i[:])
            OH = cpool.tile([128, H * 128], BF16)
            nc.vector.tensor_scalar(OH[:], oh_f[:], 0.0, None,
                                    op0=ALU.is_equal)

            w1_t = cpool.tile([128, KT * TC1], BF16)
            for kk in range(KT):
                nc.sync.dma_start(
                    w1_t[:, kk * TC1:(kk + 1) * TC1],
                    w1[kk * 128:(kk + 1) * 128, :])
            w2_t = cpool.tile([HD, TC2], BF16)
            nc.sync.dma_start(w2_t[:], w2[:])
            w3_t = cpool.tile([HD, TC3], BF16)
            nc.sync.dma_start(w3_t[:], w3[:])
            idx_t = cpool.tile([128, NCH], I32)
            nc.sync.dma_start(idx_t[:], idx32[:])
            drc_b = cpool.tile([128, NCH], BF16)
            nc.sync.dma_start(drc_b[:], drel_c[:])
            drc_t = cpool.tile([128, NCH], F32)
            nc.vector.tensor_copy(drc_t[:], drc_b[:])
            ad1_t = cpool.tile([128, NW * H], F32)
            ad2_t = cpool.tile([128, NW], F32)
            ad3_t = cpool.tile([128, NW], F32)

            bounce1 = dram.tile([NDP, TC1], BF16)
            table1 = dram.tile([NP, TC1], BF16)
            h1T = dram.tile([HD, NDP], BF16)
            bounce2 = dram.tile([NDP, TC2], BF16)
            table2 = dram.tile([NP, TC2], BF16)
            h2T = dram.tile([HD, NDP], BF16)
            bounce3 = dram.tile([NDP, TC3], BF16)
            table3 = dram.tile([NP, TC3], BF16)

            def dense(lhsT_dram, w_t, kt, tcols, xh_cols, heads, ad_t,
                      bounce):
                """xh = lhsT^T @ W_aug per 128-node tile; stage bf16 rows
                [xh | ad | as | 1] to bounce; stash ad columns (f32)."""
                for t in range(NW):
                    ps = pp.tile([128, tcols], F32, tag="big")
                    for kk in range(kt):
                        lt = dpool.tile([128, 128], BF16, tag="dlhs")
                        nc.sync.dma_start(
                            lt[:], lhsT_dram[kk * 128:(kk + 1) * 128,
                                             t * 128:(t + 1) * 128])
                        nc.tensor.matmul(
                            out=ps[:], lhsT=lt[:],
                            rhs=w_t[:, kk * tcols:(kk + 1) * tcols],
                            start=(kk == 0), stop=(kk == kt - 1))
                    xh_sb = dpool.tile([128, tcols], BF16, tag="dxh")
                    nc.vector.memset(xh_sb[:], 1.0)
                    nc.vector.tensor_copy(
                        xh_sb[:, :xh_cols + 2 * heads],
                        ps[:, :xh_cols + 2 * heads])
                    nc.sync.dma_start(bounce[t * 128:(t + 1) * 128, :],
                                      xh_sb[:])
                    nc.vector.tensor_copy(
                        ad_t[:, t * heads:(t + 1) * heads],
                        ps[:, xh_cols:xh_cols + heads])

            def edge_layer(table, tcols, xcols, heads, ad_t, out_write):
                onec = xcols + 2 * heads  # ones column index
                for w in range(NW):
                    c0 = w * CPW
                    # gathers: CPW indirect row-fetches into one wide tile
                    G = wpool.tile([128, CPW * tcols], BF16, tag="G")
                    for j in range(CPW):
                        nc.gpsimd.indirect_dma_start(
                            out=G[:, j * tcols:j * tcols + tcols],
                            out_offset=None, in_=table[:],
                            in_offset=bass.IndirectOffsetOnAxis(
                                ap=idx_t[:, c0 + j:c0 + j + 1], axis=0))
                    Gv = G[:].rearrange("p (c t) -> p c t", c=CPW)
                    # one-hot blocks for all chunks: EQ[p,c,d] =
                    #   (drel[p,c] == iota[d])
                    EQ = wpool.tile([128, CPW * 128], BF16, tag="EQ")
                    EQv = EQ[:].rearrange("p (c d) -> p c d", c=CPW)
                    eq0, eq1 = bass.broadcast_tensor_aps(
                        drc_t[:, c0:c0 + CPW].rearrange(
                            "p (c o) -> p c o", o=1),
                        iota_row[:].rearrange("p (o d) -> p o d", o=1))
                    nc.vector.tensor_tensor(
                        out=EQv, in0=eq0, in1=eq1, op=ALU.is_equal)
                    # replicate ad across partitions: adT = transpose(ad_w)
                    adp = ptp.tile([128, 128], F32, tag="adT")
                    nc.tensor.transpose(
                        out=adp[:heads, :],
                        in_=ad_t[:, w * heads:(w + 1) * heads],
                        identity=ident[:])
                    adT = spool.tile([128, 128], BF16, tag="adT")
                    nc.vector.memset(adT[:], 0.0)
                    nc.vector.tensor_copy(adT[:heads, :], adp[:heads, :])
                    # per-edge a_dst + logits, per head
                    sv = spool.tile([128, heads * CPW], F32, tag="sv")
                    junk = wpool.tile([128, CPW * 128], BF16, tag="junk")
                    junkv = junk[:].rearrange("p (c d) -> p c d", c=CPW)
                    for h in range(heads):
                        adrep = pad_p.tile([128, 128], F32, tag="adrep")
                        nc.tensor.matmul(
                            out=adrep[:], lhsT=OH[:, h * 128:(h + 1) * 128],
                            rhs=adT[:], start=True, stop=True)
                        # pade[p,c] = sum_d EQ[p,c,d] * adrep[d]
                        p0, p1 = bass.broadcast_tensor_aps(
                            EQv,
                            adrep[:].rearrange("p (o d) -> p o d", o=1))
                        nc.vector.tensor_tensor(
                            out=junkv, in0=p0, in1=p1, op=ALU.mult)
                        nc.vector.tensor_reduce(
                            out=sv[:, h * CPW:(h + 1) * CPW],
                            in_=junkv, op=ALU.add,
                            axis=mybir.AxisListType.X)
                    # + a_src columns of G (strided extract-add)
                    for h in range(heads):
                        nc.vector.tensor_tensor(
                            out=sv[:, h * CPW:(h + 1) * CPW],
                            in0=sv[:, h * CPW:(h + 1) * CPW],
                            in1=Gv[:, :, xcols + heads + h],
                            op=ALU.add)
                    # leaky relu, clamp, exp
                    ev = spool.tile([128, heads * CPW], F32, tag="ev")
                    nc.vector.tensor_scalar_mul(ev[:], sv[:], 0.2)
                    nc.vector.tensor_tensor(out=ev[:], in0=sv[:], in1=ev[:],
                                            op=ALU.max)
                    nc.vector.tensor_scalar_min(ev[:], ev[:], 60.0)
                    al = spool.tile([128, heads * CPW], F32, tag="al")
                    nc.scalar.activation(al[:], ev[:], AF.Exp)
                    # M = G * alpha (per-head broadcast over feature cols)
                    M = wpool.tile([128, CPW * tcols], BF16, tag="M")
                    Mv = M[:].rearrange("p (c t) -> p c t", c=CPW)
                    m0, m1 = bass.broadcast_tensor_aps(
                        Gv[:, :, :xcols].rearrange(
                            "p c (h f) -> p c h f", h=heads),
                        al[:].rearrange("p (h c o) -> p c h o", h=heads,
                                        o=1))
                    nc.vector.tensor_tensor(
                        out=Mv[:, :, :xcols].rearrange(
                            "p c (h f) -> p c h f", h=heads),
                        in0=m0, in1=m1, op=ALU.mult)
                    # den source: cols xcols..xcols+heads-1 := al_h
                    nc.vector.tensor_copy(
                        Mv[:, :, xcols:xcols + heads],
                        al[:].rearrange("p (h c) -> p c h", h=heads))
                    # aggregate: psw[d, :] += EQ_c^T-free matmuls
                    psw = pp.tile([128, xcols + heads], F32, tag="big")
                    for j in range(CPW):
                        nc.tensor.matmul(
                            out=psw[:],
                            lhsT=EQ[:, j * 128:(j + 1) * 128],
                            rhs=M[:, j * tcols:j * tcols + xcols + heads],
                            start=(j == 0), stop=(j == CPW - 1))
                    den = spool.tile([128, heads], F32, tag="den")
                    nc.vector.tensor_scalar_max(
                        den[:], psw[:, xcols:xcols + heads], 1e-30)
                    rden = spool.tile([128, heads], F32, tag="rden")
                    nc.vector.reciprocal(rden[:], den[:])
                    out_write(w, psw, rden)

            # ---- layer 1
            dense(xT, w1_t, KT, TC1, XC1, H, ad1_t, bounce1)
            nc.gpsimd.collective_compute(
                "AllGather", ALU.bypass, replica_groups=[list(range(NC))],
                ins=[bounce1.opt()], outs=[table1.opt()])

            def wr1(w, psw, rden):
                hsb = dpool.tile([128, HD], BF16, tag="hsb")
                for h in range(H):
                    nc.scalar.activation(hsb[:, h * HID:(h + 1) * HID],
                                         psw[:, h * HID:(h + 1) * HID],
                                         AF.Relu, scale=rden[:, h:h + 1])
                pt = ptp.tile([128, 128], BF16, tag="tps")
                nc.tensor.transpose(out=pt[:], in_=hsb[:],
                                    identity=ident_b[:])
                htt = dpool.tile([128, 128], BF16, tag="htt")
                nc.vector.tensor_copy(htt[:], pt[:])
                nc.sync.dma_start(h1T[:, w * 128:(w + 1) * 128], htt[:])

            edge_layer(table1, TC1, XC1, H, ad1_t, wr1)

            # ---- layer 2
            dense(h1T, w2_t, 1, TC2, XC2, 1, ad2_t, bounce2)
            nc.gpsimd.collective_compute(
                "AllGather", ALU.bypass, replica_groups=[list(range(NC))],
                ins=[bounce2.opt()], outs=[table2.opt()])

            def wr2(w, psw, rden):
                hsb = dpool.tile([128, HD], BF16, tag="hsb")
                nc.scalar.activation(hsb[:], psw[:, :HD], AF.Relu,
                                     scale=rden[:, 0:1])
                pt = ptp.tile([128, 128], BF16, tag="tps")
                nc.tensor.transpose(out=pt[:], in_=hsb[:],
                                    identity=ident_b[:])
                htt = dpool.tile([128, 128], BF16, tag="htt")
                nc.vector.tensor_copy(htt[:], pt[:])
                nc.sync.dma_start(h2T[:, w * 128:(w + 1) * 128], htt[:])

            edge_layer(table2, TC2, XC2, 1, ad2_t, wr2)

            # ---- layer 3
            dense(h2T, w3_t, 1, TC3, XC3, 1, ad3_t, bounce3)
            nc.gpsimd.collective_compute(
                "AllGather", ALU.bypass, replica_groups=[list(range(NC))],
                ins=[bounce3.opt()], outs=[table3.opt()])

            def wr3(w, psw, rden):
                z = dpool.tile([128, OUT], F32, tag="z")
                nc.vector.tensor_scalar_mul(z[:], psw[:, :OUT], rden[:, 0:1])
                mx = spool.tile([128, 1], F32, tag="mx")
                nc.vector.reduce_max(out=mx[:], in_=z[:], op=ALU.max,
                                     axis=mybir.AxisListType.X)
                nmx = spool.tile([128, 1], F32, tag="nmx")
                nc.vector.tensor_scalar_mul(nmx[:], mx[:], -1.0)
                ez = dpool.tile([128, OUT], F32, tag="ez")
                se = spool.tile([128, 1], F32, tag="se")
                nc.scalar.activation(ez[:], z[:], AF.Exp, bias=nmx[:],
                                     accum_out=se[:])
                ln = spool.tile([128, 1], F32, tag="ln")
                nc.scalar.activation(ln[:], se[:], AF.Ln)
                zo = dpool.tile([128, OUT], mybir.dt.float16, tag="zo")
                nc.vector.tensor_scalar(zo[:], z[:], mx[:], ln[:],
                                        op0=ALU.subtract, op1=ALU.subtract)
                nc.sync.dma_start(out[w * 128:(w + 1) * 128, :], zo[:])

            edge_layer(table3, TC3, XC3, 1, ad3_t, wr3)

    nc.compile()
    return nc


def kernel(**inputs):
    edge = np.asarray(inputs["edge"])
    x = np.asarray(inputs["features"]).astype(np.float32)
    cores, shapes = _host_prep(edge, N, NC)
    W1a, W2a, W3a = _pack_weights(
        np.asarray(inputs["W1"], np.float32),
        np.asarray(inputs["a_src1"], np.float32),
        np.asarray(inputs["a_dst1"], np.float32),
        np.asarray(inputs["W2"], np.float32),
        np.asarray(inputs["a_src2"], np.float32),
        np.asarray(inputs["a_dst2"], np.float32),
        np.asarray(inputs["W3"], np.float32),
        np.asarray(inputs["a_src3"], np.float32),
        np.asarray(inputs["a_dst3"], np.float32))
    key = (shapes["CPW"], shapes["NDP"])
    if key not in _CACHE:
        _CACHE[key] = _build_kernel(shapes)
    nc = _CACHE[key]
    ND, NDP = shapes["ND"], shapes["NDP"]
    in_maps = []
    w1u = W1a.astype(BF16NP).view(np.uint16).ravel()
    w2u = W2a.astype(BF16NP).view(np.uint16).ravel()
    w3u = W3a.astype(BF16NP).view(np.uint16).ravel()
    for k in range(NC):
        xs = np.zeros((IN, NDP), np.float32)
        xs[:, :ND] = x[k * ND:(k + 1) * ND].T
        cd = cores[k]
        blob = np.concatenate([
            xs.astype(BF16NP).view(np.uint16).ravel(),
            w1u, w2u, w3u,
            cd["drel_col"].astype(BF16NP).view(np.uint16).ravel(),
            cd["idx32"].view(np.uint16).ravel(),
        ]).reshape(128, -1)
        in_maps.append(dict(blob=blob.view(BF16NP)))
    res = bass_utils.run_bass_kernel_spmd(
        nc, in_maps, core_ids=list(range(NC)))
    outs = [res.results[k]["out"][:ND] for k in range(NC)]
    out_full = np.concatenate(outs, 0).astype(np.float32)
    return out_full
